# revision 26
# baseline (speedup 1.0000x reference)
"""Trainium2 Bass kernel for nn_ChannelLatencySeq2Seq.

Math (matching reference.py):
  - 3 depthwise convs (k=3,5,9; 6 outs each) + per-channel reduce over D=18
    collapse into ONE per-channel 9-tap FIR: Keff[c, tap]; the conv biases
    fold through the reduce into rb_eff.
  - LIF scan V_t = a*V + (1-a)*drive_t; latency = first t with V_t >= TH.
  - act = exp(-lat/scale); recon[b,j,t] = sum_c act[b,c]*G[j,c,t], where
    G = og[j,c] * sum_d fw[j,c,d]*kp[c,d,t] is nonzero only for t<9.

Sharding: data-parallel over batch B=16 across 8 cores (2 batches/core).
Per core the 512 (b,c) rows sit on 4 partition-tiles of 128; T=1024 on the
free axis.

Engine split per tile: TensorE computes 3 of the 9 FIR taps as
diagonal-stationary fp32 matmuls accumulating in PSUM (diagonals are packed
host-side into one weight-table DMA); ScalarE folds the bias while copying
PSUM->SBUF, computes Sign(V-TH), and the Exp activation; VectorE adds the
remaining 6 taps (fused scalar_tensor_tensor MACs), runs the LIF scan, and
extracts the first-crossing index with max_index against a sentinel +1
column (unfired rows naturally yield lat = T).  The reconstruction einsum
runs as a small bf16 matmul against host-packed G.
"""

import os
import sys
import numpy as np

if "/opt/trn_rl_repo" not in sys.path:
    sys.path.insert(0, "/opt/trn_rl_repo")

B, C, T = 16, 256, 1024
KERNEL_SPECS = [(3, 6), (5, 6), (9, 6)]
D = 18
TAU = 5.0
ALPHA = float(np.exp(-1.0 / TAU))
THRESHOLD = 0.01
NCORES = 8
BL = B // NCORES          # batches per core = 2
ROWS = BL * C             # 512 rows per core
NTILES = ROWS // 128      # 4
KT = 9                    # effective taps
PAD = 4
JT = C * KT               # 2304 recon columns (j major, t minor)
PE_TILES = [2, 1, 3]      # tile 0 runs fully on VectorE so it starts first
PE_TAPS = [5, 6, 7, 8]    # taps computed on TensorE for PE_TILES
NPE = len(PE_TAPS)

# packed weight-table layout (free-dim offsets), one [128, WTAB] DMA
OFF_KEFF = 0                          # per tile: 9 keff + 1 rb -> 40 cols
OFF_DIAG = NTILES * (KT + 1)          # (NTILES-1)*NPE diag blocks of 128
OFF_SC = OFF_DIAG + (NTILES - 1) * NPE * 128
OFF_ONES = OFF_SC + 1
WTAB = OFF_ONES + 8

_compiled = None
last_results = None       # BassKernelResults of most recent run (for test.py)


def _build():
    import concourse.bass as bass
    import concourse.mybir as mybir
    from concourse import bacc
    from concourse.tile import TileContext

    LEVEL = int(os.environ.get("DEBUG_LEVEL", "5"))
    f32 = mybir.dt.float32
    bf16 = mybir.dt.bfloat16
    u32 = mybir.dt.uint32
    Alu = mybir.AluOpType
    Act = mybir.ActivationFunctionType

    nc = bacc.Bacc(None, target_bir_lowering=False)

    x_ext = nc.declare_dram_parameter("x", [ROWS, T], f32, isOutput=False)
    w_ext = nc.declare_dram_parameter("wtab", [128, WTAB], f32, isOutput=False)
    g_ext = nc.declare_dram_parameter("g", [C, JT], bf16, isOutput=False)

    lat_ext = nc.declare_dram_parameter("lat_o", [128, NTILES], f32, isOutput=True)
    act_ext = nc.declare_dram_parameter("act_o", [2, 128, BL], f32, isOutput=True)
    recon_ext = nc.declare_dram_parameter("recon_o", [BL, JT], f32, isOutput=True)

    with TileContext(nc) as tc:
        with (
            tc.tile_pool(name="const", bufs=1) as const,
            tc.tile_pool(name="work", bufs=2) as work,
            tc.tile_pool(name="cpsum", bufs=1, space="PSUM") as cpsum,
            tc.tile_pool(name="rpsum", bufs=1, space="PSUM") as rpsum,
        ):
            # ---- input DMAs: x0, wtab, x1..x3 on Sync; G on the ACT queue ----
            xpads = []
            for i in range(NTILES):
                xpad = work.tile([128, T + 2 * PAD], f32, tag=f"xpad{i}",
                                 name=f"xpad{i}", bufs=1)
                if i == 0:
                    wtab = const.tile([128, WTAB], f32, tag="wtab")
                    nc.sync.dma_start(out=wtab[:, :], in_=w_ext[:, :])
                nc.sync.dma_start(out=xpad[:, PAD:PAD + T],
                                  in_=x_ext[128 * i:128 * (i + 1), :])
                nc.gpsimd.memset(xpad[:, 0:PAD], 0.0)
                nc.gpsimd.memset(xpad[:, T + PAD:T + 2 * PAD], 0.0)
                xpads.append(xpad)

            g_sb = []
            for h in range(2):
                gt = const.tile([128, JT], bf16, tag=f"g{h}", name=f"g{h}")
                nc.scalar.dma_start(out=gt[:, :], in_=g_ext[128 * h:128 * (h + 1), :])
                g_sb.append(gt)

            def keff_col(i, tap):
                base = i * (KT + 1)
                return wtab[:, base + tap:base + tap + 1]

            def rb_col(i):
                base = i * (KT + 1)
                return wtab[:, base + KT:base + KT + 1]

            def diag_blk(i, ti):
                base = OFF_DIAG + ((i - 1) * NPE + ti) * 128
                return wtab[:, base:base + 128]

            scT = wtab[:, OFF_SC:OFF_SC + 1]
            ones8 = wtab[:, OFF_ONES:OFF_ONES + 8]

            thT = const.tile([128, 1], f32, tag="thT")
            nc.gpsimd.memset(thT[:, :], -THRESHOLD)
            alphaT = const.tile([128, T], f32, tag="alphaT")
            nc.gpsimd.memset(alphaT[:, :], ALPHA)

            lat_pack = const.tile([128, NTILES], f32, tag="lat_pack")
            actF = [const.tile([128, BL], f32, tag=f"actF{h}", name=f"actF{h}") for h in range(2)]
            actB = [const.tile([128, BL], bf16, tag=f"actB{h}", name=f"actB{h}") for h in range(2)]

            # sentinel tiles: col T holds +1 so max_index yields T for unfired rows
            sgns = []
            for i in range(NTILES):
                sg = const.tile([128, T + 8], f32, tag=f"sgn{i}", name=f"sgn{i}")
                nc.gpsimd.memset(sg[:, T:T + 8], 1.0)
                sgns.append(sg)

            # prewarm ACT function tables during the initial DMA wait
            warm = const.tile([128, 1], f32, tag="warm")
            nc.scalar.activation(out=warm[:, :], in_=thT[:, :], func=Act.Sign)
            nc.scalar.activation(out=warm[:, :], in_=thT[:, :], func=Act.Exp)

            rchunks = [(n0, min(512, JT - n0)) for n0 in range(0, JT, 512)]
            rps = [rpsum.tile([BL, nn], f32, tag=f"rps{ci}", name=f"rps{ci}")
                   for ci, (n0, nn) in enumerate(rchunks)]

            # ---- phase A: PE taps + bias copies for tiles 2,1,3 ----
            # Trace order keeps each engine's in-order stream stall-free: all
            # Identity copies precede all Signs on ScalarE.
            parts = {}
            for i in PE_TILES:
                xpad = xpads[i]
                pconv = cpsum.tile([128, T], f32, tag="pconv", name=f"pconv{i}")
                for chunk in range(2):
                    n0 = 512 * chunk
                    for ti, tap in enumerate(PE_TAPS):
                        nc.tensor.matmul(
                            out=pconv[:, n0:n0 + 512], lhsT=diag_blk(i, ti),
                            rhs=xpad[:, tap + n0:tap + n0 + 512],
                            start=(ti == 0), stop=(ti == NPE - 1))
                part = work.tile([128, T], f32, tag="part", name=f"part{i}")
                for chunk in range(2):
                    n0 = 512 * chunk
                    nc.scalar.activation(out=part[:, n0:n0 + 512], in_=pconv[:, n0:n0 + 512],
                                         func=Act.Identity, bias=rb_col(i), scale=1.0)
                parts[i] = part

            # ---- phase B: VectorE taps + scans; Signs/FINDs woven in ----
            vts = {}
            for i in [0, 2, 1, 3]:
                xpad = xpads[i]
                drive = work.tile([128, T], f32, tag="drive", name=f"drive{i}")
                if i == 0:
                    nc.vector.tensor_scalar(
                        out=drive[:, :], in0=xpad[:, 0:T],
                        scalar1=keff_col(i, 0), scalar2=rb_col(i),
                        op0=Alu.mult, op1=Alu.add)
                    rest = range(1, KT)
                else:
                    nc.vector.scalar_tensor_tensor(
                        out=drive[:, :], in0=xpad[:, 0:T],
                        scalar=keff_col(i, 0), in1=parts[i][:, :],
                        op0=Alu.mult, op1=Alu.add)
                    rest = [t for t in range(1, KT) if t not in PE_TAPS]
                for tap in rest:
                    nc.vector.scalar_tensor_tensor(
                        out=drive[:, :], in0=xpad[:, tap:tap + T],
                        scalar=keff_col(i, tap), in1=drive[:, :],
                        op0=Alu.mult, op1=Alu.add)
                if LEVEL < 3:
                    nc.vector.tensor_scalar(
                        out=lat_pack[:, i:i + 1], in0=drive[:, 100:101],
                        scalar1=1.0, scalar2=None, op0=Alu.mult)
                    continue
                vtile = work.tile([128, T], f32, tag="vtile", name=f"vtile{i}")
                nc.vector.tensor_tensor_scan(
                    out=vtile[:, :], data0=alphaT[:, :], data1=drive[:, :],
                    initial=0.0, op0=Alu.mult, op1=Alu.add)
                vts[i] = vtile
                # Sign on ScalarE as soon as the scan lands
                nc.scalar.activation(
                    out=sgns[i][:, 0:T], in_=vts[i][:, :], func=Act.Sign,
                    bias=thT[:, 0:1], scale=1.0)

            if LEVEL >= 3:
                # FIND + cast + exp per tile; recon h0 after tiles 0,2 are in
                def finish(i):
                    idx = work.tile([128, 8], u32, tag="idx", name=f"idx{i}")
                    nc.vector.max_index(idx[:, :], ones8, sgns[i][:, 0:T + 1])
                    nc.vector.tensor_scalar(
                        out=lat_pack[:, i:i + 1], in0=idx[:, 0:1],
                        scalar1=1.0, scalar2=None, op0=Alu.mult)
                    h, b = i % 2, i // 2
                    nc.scalar.activation(
                        out=actF[h][:, b:b + 1], in_=lat_pack[:, i:i + 1],
                        func=Act.Exp, bias=0.0, scale=scT)
                    nc.scalar.activation(
                        out=actB[h][:, b:b + 1], in_=lat_pack[:, i:i + 1],
                        func=Act.Exp, bias=0.0, scale=scT)

                finish(0)
                finish(2)
                if LEVEL >= 5:
                    for ci, (n0, nn) in enumerate(rchunks):
                        nc.tensor.matmul(
                            out=rps[ci][:, :], lhsT=actB[0][:, :],
                            rhs=g_sb[0][:, n0:n0 + nn],
                            start=True, stop=False, skip_group_check=True)
                finish(1)
                finish(3)

            # ---- recon finish: half h=1, then DMA straight from PSUM ----
            if LEVEL >= 5:
                for ci, (n0, nn) in enumerate(rchunks):
                    nc.tensor.matmul(
                        out=rps[ci][:, :], lhsT=actB[1][:, :],
                        rhs=g_sb[1][:, n0:n0 + nn],
                        start=False, stop=True, skip_group_check=True)
                recon_sb = const.tile([BL, JT], f32, tag="recon_sb")
                for ci, (n0, nn) in enumerate(rchunks):
                    # split the PSUM->SBUF copies across VectorE and ScalarE
                    if ci % 2 == 0:
                        nc.vector.tensor_copy(out=recon_sb[:, n0:n0 + nn], in_=rps[ci][:, :])
                    else:
                        nc.scalar.activation(out=recon_sb[:, n0:n0 + nn], in_=rps[ci][:, :],
                                             func=Act.Copy, bias=0.0, scale=1.0)
                nc.sync.dma_start(out=recon_ext[:, :], in_=recon_sb[:, :])
            else:
                zr = const.tile([BL, JT], f32, tag="zr")
                nc.vector.memset(zr[:, :], 0.0)
                nc.sync.dma_start(out=recon_ext[:, :], in_=zr[:, :])
                for h in range(2):
                    nc.vector.memset(actF[h][:, :], 0.0)

            # ---- outputs ----
            nc.sync.dma_start(out=lat_ext[:, :], in_=lat_pack[:, :])
            for h in range(2):
                nc.sync.dma_start(out=act_ext[h], in_=actF[h][:, :])

    nc.compile()
    return nc


def _host_prep(inputs):
    """Host-side packing of weight-derived constants (no x-dependent math)."""
    import ml_dtypes
    gi = lambda k: np.asarray(inputs[k], np.float32)
    x = gi("x")
    rw = gi("reduce_w")            # (C, D)
    rbv = gi("reduce_b")           # (C,)
    og = gi("output_gates")        # (C, C)
    fw = gi("filter_weights")      # (C, C, D)
    ls = float(np.asarray(inputs["latency_scale"], np.float32))

    ws = {k: gi(f"w{k}").reshape(C, op, k) for k, op in KERNEL_SPECS}

    # Keff[c, tap]  (tap index 0..8 maps to time offset tap-4)
    keff = np.zeros((C, KT), np.float64)
    off = 0
    for k, op in KERNEL_SPECS:
        p = (k - 1) // 2
        w = ws[k].astype(np.float64)                      # (C, op, k)
        for i in range(k):
            tap = i - p + PAD
            keff[:, tap] += (w[:, :, i] * rw[:, off:off + op].astype(np.float64)).sum(axis=1)
        off += op
    keff *= (1.0 - ALPHA)
    keff = keff.astype(np.float32)                        # (C, 9)

    # conv biases flow through the reduce einsum: rb_eff = reduce_b + sum_d rw*b_d
    biases = np.concatenate(
        [np.asarray(inputs[f"b{k}"], np.float32).reshape(C, op) for k, op in KERNEL_SPECS],
        axis=1)                                           # (C, D)
    rb_eff = rbv.astype(np.float64) + (rw.astype(np.float64) * biases.astype(np.float64)).sum(axis=1)
    rb2 = ((1.0 - ALPHA) * rb_eff).astype(np.float32)     # (C,)

    # G[c, j*9+t] = og[j,c] * sum_d fw[j,c,d] * kp[c,d,t]
    kp = np.zeros((C, D, KT), np.float64)
    off = 0
    for k, op in KERNEL_SPECS:
        kp[:, off:off + op, :k] = ws[k].astype(np.float64)
        off += op
    gjct = np.einsum("jcd,cdt->jct", og.astype(np.float64)[:, :, None] * fw.astype(np.float64), kp)
    gmat = np.ascontiguousarray(gjct.transpose(1, 0, 2).reshape(C, JT)).astype(ml_dtypes.bfloat16)

    scale = max(ls, 0.001)

    keff_rows = np.tile(keff, (BL, 1))                     # (512, 9)
    rb_rows = np.tile(rb2.reshape(C, 1), (BL, 1))          # (512, 1)

    # packed weight table [128, WTAB]
    wtab = np.zeros((128, WTAB), np.float32)
    for i in range(NTILES):
        base = i * (KT + 1)
        wtab[:, base:base + KT] = keff_rows[128 * i:128 * (i + 1)]
        wtab[:, base + KT] = rb_rows[128 * i:128 * (i + 1), 0]
        for ti, tap in enumerate(PE_TAPS):
            if i == 0:
                continue
            blk = OFF_DIAG + ((i - 1) * NPE + ti) * 128
            np.fill_diagonal(wtab[:, blk:blk + 128], keff_rows[128 * i:128 * (i + 1), tap])
    wtab[:, OFF_SC] = -1.0 / scale
    wtab[:, OFF_ONES:OFF_ONES + 8] = 1.0

    in_maps = []
    for core in range(NCORES):
        xs = np.ascontiguousarray(x[BL * core: BL * (core + 1)].reshape(ROWS, T))
        in_maps.append(dict(x=xs, wtab=wtab, g=gmat))
    return in_maps


def kernel(**inputs):
    global _compiled, last_results
    from concourse.bass_utils import run_bass_kernel_spmd

    x = np.asarray(inputs["x"], np.float32)
    in_maps = _host_prep(inputs)

    if _compiled is None:
        _compiled = _build()
    nc = _compiled

    res = run_bass_kernel_spmd(nc, in_maps, list(range(NCORES)))
    last_results = res

    lat = np.empty((B, C), np.float32)
    act = np.empty((B, C), np.float32)
    recon = np.zeros((B, C, T), np.float32)
    for core in range(NCORES):
        r = res.results[core]
        lat_np = r["lat_o"]                      # (128, 4): col i = tile i
        act_np = r["act_o"]                      # (2, 128, BL)
        rec_np = r["recon_o"]                    # (BL, JT)
        lat_rows = lat_np.T.reshape(ROWS)        # rows (b*C + c)
        lat[BL * core: BL * (core + 1)] = lat_rows.reshape(BL, C)
        for b in range(BL):
            for h in range(2):
                act[BL * core + b, 128 * h:128 * (h + 1)] = act_np[h, :, b]
        recon[BL * core: BL * (core + 1), :, :KT] = rec_np.reshape(BL, C, KT)
    return recon, x, lat, act


# revision 28
# speedup vs baseline: 1.0719x; 1.0719x over previous
"""Trainium2 Bass kernel for nn_ChannelLatencySeq2Seq.

Math (matching reference.py):
  - 3 depthwise convs (k=3,5,9; 6 outs each) + per-channel reduce over D=18
    collapse into ONE per-channel 9-tap FIR: Keff[c, tap]; the conv biases
    fold through the reduce into rb_eff.
  - LIF scan V_t = a*V + (1-a)*drive_t; latency = first t with V_t >= TH.
  - act = exp(-lat/scale); recon[b,j,t] = sum_c act[b,c]*G[j,c,t], where
    G = og[j,c] * sum_d fw[j,c,d]*kp[c,d,t] is nonzero only for t<9.

Sharding: data-parallel over batch B=16 across 8 cores (2 batches/core).
Per core the 512 (b,c) rows sit on 4 partition-tiles of 128; T=1024 on the
free axis.

Engine split per tile: TensorE computes 3 of the 9 FIR taps as
diagonal-stationary fp32 matmuls accumulating in PSUM (diagonals are packed
host-side into one weight-table DMA); ScalarE folds the bias while copying
PSUM->SBUF, computes Sign(V-TH), and the Exp activation; VectorE adds the
remaining 6 taps (fused scalar_tensor_tensor MACs), runs the LIF scan, and
extracts the first-crossing index with max_index against a sentinel +1
column (unfired rows naturally yield lat = T).  The reconstruction einsum
runs as a small bf16 matmul against host-packed G.
"""

import os
import sys
import numpy as np

if "/opt/trn_rl_repo" not in sys.path:
    sys.path.insert(0, "/opt/trn_rl_repo")

B, C, T = 16, 256, 1024
KERNEL_SPECS = [(3, 6), (5, 6), (9, 6)]
D = 18
TAU = 5.0
ALPHA = float(np.exp(-1.0 / TAU))
THRESHOLD = 0.01
NCORES = 8
BL = B // NCORES          # batches per core = 2
ROWS = BL * C             # 512 rows per core
NTILES = ROWS // 128      # 4
KT = 9                    # effective taps
PAD = 4
JT = C * KT               # 2304 recon columns (j major, t minor)
PE_TILES = [2, 1, 3]      # tile 0 runs fully on VectorE so it starts first
PE_TAPS = [5, 6, 7, 8]    # taps computed on TensorE for PE_TILES
NPE = len(PE_TAPS)

# packed weight tables: small (keff/rb/sc/ths8) and diag stationaries
OFF_KEFF = 0                          # per tile: 9 keff + 1 rb -> 40 cols
OFF_SC = NTILES * (KT + 1)
OFF_THS = OFF_SC + 1
WSMALL = OFF_THS + 8
WDIAG = (NTILES - 1) * NPE * 128

_compiled = None
last_results = None       # BassKernelResults of most recent run (for test.py)


def _build():
    import concourse.bass as bass
    import concourse.mybir as mybir
    from concourse import bacc
    from concourse.tile import TileContext

    LEVEL = int(os.environ.get("DEBUG_LEVEL", "5"))
    f32 = mybir.dt.float32
    bf16 = mybir.dt.bfloat16
    u32 = mybir.dt.uint32
    Alu = mybir.AluOpType
    Act = mybir.ActivationFunctionType

    nc = bacc.Bacc(None, target_bir_lowering=False)

    x_ext = nc.declare_dram_parameter("x", [ROWS, T], f32, isOutput=False)
    w_ext = nc.declare_dram_parameter("wtab", [128, WSMALL], f32, isOutput=False)
    wd_ext = nc.declare_dram_parameter("wdiag", [128, WDIAG], f32, isOutput=False)
    g_ext = nc.declare_dram_parameter("g", [C, JT], bf16, isOutput=False)

    lat_ext = nc.declare_dram_parameter("lat_o", [128, NTILES], f32, isOutput=True)
    act_ext = nc.declare_dram_parameter("act_o", [2, 128, BL], f32, isOutput=True)
    recon_ext = nc.declare_dram_parameter("recon_o", [BL, JT], f32, isOutput=True)

    with TileContext(nc) as tc:
        with (
            tc.tile_pool(name="const", bufs=1) as const,
            tc.tile_pool(name="work", bufs=2) as work,
            tc.tile_pool(name="cpsum", bufs=1, space="PSUM") as cpsum,
            tc.tile_pool(name="rpsum", bufs=1, space="PSUM") as rpsum,
        ):
            # ---- input DMAs: x0, wtab, x1..x3 on Sync; G on the ACT queue ----
            xpads = []
            for i in range(NTILES):
                xpad = work.tile([128, T + 2 * PAD], f32, tag=f"xpad{i}",
                                 name=f"xpad{i}", bufs=1)
                if i == 0:
                    wtab = const.tile([128, WSMALL], f32, tag="wtab")
                    nc.gpsimd.dma_start(out=wtab[:, :], in_=w_ext[:, :])
                    wdiag = const.tile([128, WDIAG], f32, tag="wdiag")
                    nc.gpsimd.dma_start(out=wdiag[:, :], in_=wd_ext[:, :])
                # x tiles split across the SP and ACT DMA queues
                if i < 2:
                    nc.sync.dma_start(out=xpad[:, PAD:PAD + T],
                                      in_=x_ext[128 * i:128 * (i + 1), :])
                else:
                    nc.scalar.dma_start(out=xpad[:, PAD:PAD + T],
                                        in_=x_ext[128 * i:128 * (i + 1), :])
                nc.gpsimd.memset(xpad[:, 0:PAD], 0.0)
                nc.gpsimd.memset(xpad[:, T + PAD:T + 2 * PAD], 0.0)
                xpads.append(xpad)

            g_sb = []
            for h in range(2):
                gt = const.tile([128, JT], bf16, tag=f"g{h}", name=f"g{h}")
                nc.scalar.dma_start(out=gt[:, :], in_=g_ext[128 * h:128 * (h + 1), :])
                g_sb.append(gt)

            def keff_col(i, tap):
                base = i * (KT + 1)
                return wtab[:, base + tap:base + tap + 1]

            def rb_col(i):
                base = i * (KT + 1)
                return wtab[:, base + KT:base + KT + 1]

            def diag_blk(i, ti):
                base = ((i - 1) * NPE + ti) * 128
                return wdiag[:, base:base + 128]

            scT = wtab[:, OFF_SC:OFF_SC + 1]
            ths8 = wtab[:, OFF_THS:OFF_THS + 8]

            alphaT = const.tile([128, T], f32, tag="alphaT")
            nc.gpsimd.memset(alphaT[:, :], ALPHA)

            lat_pack = const.tile([128, NTILES], f32, tag="lat_pack")
            actF = [const.tile([128, BL], f32, tag=f"actF{h}", name=f"actF{h}") for h in range(2)]
            actB = [const.tile([128, BL], bf16, tag=f"actB{h}", name=f"actB{h}") for h in range(2)]

            # z = min(V, TH) tiles; col T holds the TH sentinel so max_index
            # (hunting the value TH) yields T for unfired rows
            sgns = []
            for i in range(NTILES):
                sg = const.tile([128, T + 8], f32, tag=f"sgn{i}", name=f"sgn{i}")
                nc.gpsimd.memset(sg[:, T:T + 8], THRESHOLD)
                sgns.append(sg)

            # prewarm the ACT Exp table during the initial DMA wait
            warm = const.tile([128, 1], f32, tag="warm")
            nc.gpsimd.memset(warm[:, :], 0.0)
            nc.scalar.activation(out=warm[:, :], in_=warm[:, :], func=Act.Exp)

            rchunks = [(n0, min(512, JT - n0)) for n0 in range(0, JT, 512)]
            rps = [rpsum.tile([BL, nn], f32, tag=f"rps{ci}", name=f"rps{ci}")
                   for ci, (n0, nn) in enumerate(rchunks)]

            # ---- phase A: PE taps + bias copies for tiles 2,1,3 ----
            # Trace order keeps each engine's in-order stream stall-free: all
            # Identity copies precede all Signs on ScalarE.
            parts = {}
            for i in PE_TILES:
                xpad = xpads[i]
                pconv = cpsum.tile([128, T], f32, tag="pconv", name=f"pconv{i}")
                for chunk in range(2):
                    n0 = 512 * chunk
                    for ti, tap in enumerate(PE_TAPS):
                        nc.tensor.matmul(
                            out=pconv[:, n0:n0 + 512], lhsT=diag_blk(i, ti),
                            rhs=xpad[:, tap + n0:tap + n0 + 512],
                            start=(ti == 0), stop=(ti == NPE - 1))
                part = work.tile([128, T], f32, tag="part", name=f"part{i}")
                for chunk in range(2):
                    n0 = 512 * chunk
                    nc.scalar.activation(out=part[:, n0:n0 + 512], in_=pconv[:, n0:n0 + 512],
                                         func=Act.Identity, bias=rb_col(i), scale=1.0)
                parts[i] = part

            # ---- phase B: VectorE taps + scans; Signs/FINDs woven in ----
            vts = {}
            for i in [0, 2, 1, 3]:
                xpad = xpads[i]
                drive = work.tile([128, T], f32, tag="drive", name=f"drive{i}")
                if i == 0:
                    nc.vector.tensor_scalar(
                        out=drive[:, :], in0=xpad[:, 0:T],
                        scalar1=keff_col(i, 0), scalar2=rb_col(i),
                        op0=Alu.mult, op1=Alu.add)
                    rest = range(1, KT)
                else:
                    nc.vector.scalar_tensor_tensor(
                        out=drive[:, :], in0=xpad[:, 0:T],
                        scalar=keff_col(i, 0), in1=parts[i][:, :],
                        op0=Alu.mult, op1=Alu.add)
                    rest = [t for t in range(1, KT) if t not in PE_TAPS]
                for tap in rest:
                    nc.vector.scalar_tensor_tensor(
                        out=drive[:, :], in0=xpad[:, tap:tap + T],
                        scalar=keff_col(i, tap), in1=drive[:, :],
                        op0=Alu.mult, op1=Alu.add)
                if LEVEL < 3:
                    nc.vector.tensor_scalar(
                        out=lat_pack[:, i:i + 1], in0=drive[:, 100:101],
                        scalar1=1.0, scalar2=None, op0=Alu.mult)
                    continue
                vtile = work.tile([128, T], f32, tag="vtile", name=f"vtile{i}")
                nc.vector.tensor_tensor_scan(
                    out=vtile[:, :], data0=alphaT[:, :], data1=drive[:, :],
                    initial=0.0, op0=Alu.mult, op1=Alu.add)
                vts[i] = vtile
                # z = min(V, TH): stays on VectorE (2x single-src mode)
                nc.vector.tensor_scalar(
                    out=sgns[i][:, 0:T], in0=vtile[:, :],
                    scalar1=THRESHOLD, scalar2=None, op0=Alu.min)

            if LEVEL >= 3:
                # FIND + cast + exp per tile; recon h0 after tiles 0,2 are in
                def finish(i):
                    idx = work.tile([128, 8], u32, tag="idx", name=f"idx{i}")
                    nc.vector.max_index(idx[:, :], ths8, sgns[i][:, 0:T + 1])
                    nc.vector.tensor_scalar(
                        out=lat_pack[:, i:i + 1], in0=idx[:, 0:1],
                        scalar1=1.0, scalar2=None, op0=Alu.mult)
                    h, b = i % 2, i // 2
                    nc.scalar.activation(
                        out=actF[h][:, b:b + 1], in_=lat_pack[:, i:i + 1],
                        func=Act.Exp, bias=0.0, scale=scT)
                    nc.scalar.activation(
                        out=actB[h][:, b:b + 1], in_=lat_pack[:, i:i + 1],
                        func=Act.Exp, bias=0.0, scale=scT)

                finish(0)
                finish(2)
                if LEVEL >= 5:
                    for ci, (n0, nn) in enumerate(rchunks):
                        nc.tensor.matmul(
                            out=rps[ci][:, :], lhsT=actB[0][:, :],
                            rhs=g_sb[0][:, n0:n0 + nn],
                            start=True, stop=False, skip_group_check=True)
                finish(1)
                finish(3)

            # ---- recon finish: half h=1, then DMA straight from PSUM ----
            if LEVEL >= 5:
                for ci, (n0, nn) in enumerate(rchunks):
                    nc.tensor.matmul(
                        out=rps[ci][:, :], lhsT=actB[1][:, :],
                        rhs=g_sb[1][:, n0:n0 + nn],
                        start=False, stop=True, skip_group_check=True)
                recon_sb = const.tile([BL, JT], f32, tag="recon_sb")
                for ci, (n0, nn) in enumerate(rchunks):
                    # split the PSUM->SBUF copies across VectorE and ScalarE
                    if ci % 2 == 0:
                        nc.vector.tensor_copy(out=recon_sb[:, n0:n0 + nn], in_=rps[ci][:, :])
                    else:
                        nc.scalar.activation(out=recon_sb[:, n0:n0 + nn], in_=rps[ci][:, :],
                                             func=Act.Copy, bias=0.0, scale=1.0)
                nc.sync.dma_start(out=recon_ext[:, :], in_=recon_sb[:, :])
            else:
                zr = const.tile([BL, JT], f32, tag="zr")
                nc.vector.memset(zr[:, :], 0.0)
                nc.sync.dma_start(out=recon_ext[:, :], in_=zr[:, :])
                for h in range(2):
                    nc.vector.memset(actF[h][:, :], 0.0)

            # ---- outputs ----
            nc.sync.dma_start(out=lat_ext[:, :], in_=lat_pack[:, :])
            for h in range(2):
                nc.sync.dma_start(out=act_ext[h], in_=actF[h][:, :])

    nc.compile()
    return nc


def _host_prep(inputs):
    """Host-side packing of weight-derived constants (no x-dependent math)."""
    import ml_dtypes
    gi = lambda k: np.asarray(inputs[k], np.float32)
    x = gi("x")
    rw = gi("reduce_w")            # (C, D)
    rbv = gi("reduce_b")           # (C,)
    og = gi("output_gates")        # (C, C)
    fw = gi("filter_weights")      # (C, C, D)
    ls = float(np.asarray(inputs["latency_scale"], np.float32))

    ws = {k: gi(f"w{k}").reshape(C, op, k) for k, op in KERNEL_SPECS}

    # Keff[c, tap]  (tap index 0..8 maps to time offset tap-4)
    keff = np.zeros((C, KT), np.float64)
    off = 0
    for k, op in KERNEL_SPECS:
        p = (k - 1) // 2
        w = ws[k].astype(np.float64)                      # (C, op, k)
        for i in range(k):
            tap = i - p + PAD
            keff[:, tap] += (w[:, :, i] * rw[:, off:off + op].astype(np.float64)).sum(axis=1)
        off += op
    keff *= (1.0 - ALPHA)
    keff = keff.astype(np.float32)                        # (C, 9)

    # conv biases flow through the reduce einsum: rb_eff = reduce_b + sum_d rw*b_d
    biases = np.concatenate(
        [np.asarray(inputs[f"b{k}"], np.float32).reshape(C, op) for k, op in KERNEL_SPECS],
        axis=1)                                           # (C, D)
    rb_eff = rbv.astype(np.float64) + (rw.astype(np.float64) * biases.astype(np.float64)).sum(axis=1)
    rb2 = ((1.0 - ALPHA) * rb_eff).astype(np.float32)     # (C,)

    # G[c, j*9+t] = og[j,c] * sum_d fw[j,c,d] * kp[c,d,t]
    kp = np.zeros((C, D, KT), np.float64)
    off = 0
    for k, op in KERNEL_SPECS:
        kp[:, off:off + op, :k] = ws[k].astype(np.float64)
        off += op
    gjct = np.einsum("jcd,cdt->jct", og.astype(np.float64)[:, :, None] * fw.astype(np.float64), kp)
    gmat = np.ascontiguousarray(gjct.transpose(1, 0, 2).reshape(C, JT)).astype(ml_dtypes.bfloat16)

    scale = max(ls, 0.001)

    keff_rows = np.tile(keff, (BL, 1))                     # (512, 9)
    rb_rows = np.tile(rb2.reshape(C, 1), (BL, 1))          # (512, 1)

    # packed weight tables
    wtab = np.zeros((128, WSMALL), np.float32)
    wdiag = np.zeros((128, WDIAG), np.float32)
    for i in range(NTILES):
        base = i * (KT + 1)
        wtab[:, base:base + KT] = keff_rows[128 * i:128 * (i + 1)]
        wtab[:, base + KT] = rb_rows[128 * i:128 * (i + 1), 0]
        for ti, tap in enumerate(PE_TAPS):
            if i == 0:
                continue
            blk = ((i - 1) * NPE + ti) * 128
            np.fill_diagonal(wdiag[:, blk:blk + 128], keff_rows[128 * i:128 * (i + 1), tap])
    wtab[:, OFF_SC] = -1.0 / scale
    wtab[:, OFF_THS:OFF_THS + 8] = THRESHOLD

    in_maps = []
    for core in range(NCORES):
        xs = np.ascontiguousarray(x[BL * core: BL * (core + 1)].reshape(ROWS, T))
        in_maps.append(dict(x=xs, wtab=wtab, wdiag=wdiag, g=gmat))
    return in_maps


def kernel(**inputs):
    global _compiled, last_results
    from concourse.bass_utils import run_bass_kernel_spmd

    x = np.asarray(inputs["x"], np.float32)
    in_maps = _host_prep(inputs)

    if _compiled is None:
        _compiled = _build()
    nc = _compiled

    res = run_bass_kernel_spmd(nc, in_maps, list(range(NCORES)))
    last_results = res

    lat = np.empty((B, C), np.float32)
    act = np.empty((B, C), np.float32)
    recon = np.zeros((B, C, T), np.float32)
    for core in range(NCORES):
        r = res.results[core]
        lat_np = r["lat_o"]                      # (128, 4): col i = tile i
        act_np = r["act_o"]                      # (2, 128, BL)
        rec_np = r["recon_o"]                    # (BL, JT)
        lat_rows = lat_np.T.reshape(ROWS)        # rows (b*C + c)
        lat[BL * core: BL * (core + 1)] = lat_rows.reshape(BL, C)
        for b in range(BL):
            for h in range(2):
                act[BL * core + b, 128 * h:128 * (h + 1)] = act_np[h, :, b]
        recon[BL * core: BL * (core + 1), :, :KT] = rec_np.reshape(BL, C, KT)
    return recon, x, lat, act
